# revision 31
# baseline (speedup 1.0000x reference)
"""Multi-head attention (B=2, D=2048, N=1024, H=16) on 8 TRN2 NeuronCores.

Sharding: batch*heads across cores - core c handles batch c//4, heads
4*(c%4) .. 4*(c%4)+3. No collectives.

Core structure (v3 - engine-balanced, software-pipelined flash):
  * Heads processed in PARITY PAIRS (A=2p at PE rows 0-63, B=2p+1 at rows
    64-127). The two K=64 score matmuls of a pair are emitted adjacently
    with disjoint PE row groups (tile_position auto-derived from
    base_partition), so they stream CONCURRENTLY through the top/bottom
    halves of the systolic array.
  * exp(S) is the 2nd bottleneck (16.8M elem/core; ScalarE alone=147us).
    Score tiles are [128,512] halves; each half's exp goes to ScalarE
    (exact ACTIVATE Exp) or VectorE (Schraudolph bit-trick + quadratic
    mantissa correction; ~0.9% max elem err, ~2.8e-3 softmax out err):
      op1: i32 = int32(S * 2^23*log2e + beta)    (tensor_scalar, PSUM->SBUF)
      op2: u = or(and(bits,MANT), 1.0);
           bf16 out = x * (1 + u*(c1 + u*c2))    (custom DVE op, SBUF->SBUF)
    Assignment: A-half of odd steps -> DVE (64 of 256 halves), which
    balances both engines at ~108us given DVE also does all evacuations.
  * Software pipeline: QK pair of step s+2 is emitted before PV of step
    s, so the PE never head-of-line blocks on the current exp; st pool
    holds 4 half-banks = 2 steps of lookahead.
  * Flash loop per (pair, iq 512-col i-chunk), j over 16 key tiles; PV
    accumulates [65,512] per head ([v | 1] weight: row 64 = denominator).
  * Projection: only (q-pair0 s0, k-pair0 s0) run up front - as a
    k-outer paired chain that finishes with the x DMA tail, so attention
    starts right at DMA-end (~40us). All remaining projection chains are
    slotted as fillers between early attention steps, backfilling the
    PE's exp-wait gaps.

Host post-pass: divide by denominator row, add the (linearly separable)
v bias, transpose + reshape into the reference's raw (B,H,D,p)->(B,D,N)
layout.
"""
import sys

sys.path.insert(0, "/opt/trn_rl_repo")

import numpy as np
import ml_dtypes
import concourse.bacc as bacc
import concourse.mybir as mybir
from concourse import tile
from concourse.bass_utils import run_bass_kernel_spmd

B, D, N, H, P = 2, 2048, 1024, 16, 64
NCORES = 8
HPC = 4            # heads per core
NPAIR = 2          # head pairs per core
KT = 8             # contraction tiles (N / 128)
JT = 16            # j (key) tiles of 128 per head
IQ = 4             # i-chunks of 512 per head
LOOK = 2           # QK leads PV by this many steps
F32R = mybir.dt.float32r
F32 = mybir.dt.float32
I32 = mybir.dt.int32
BF16 = mybir.dt.bfloat16
F16 = mybir.dt.float16
EXP = mybir.ActivationFunctionType.Exp

PJ_DT = F32R       # projection operands (x, W) on the PE. f16 PE operands
                   # were measured (v6) to downclock the whole chip ~17%
                   # (P0 power state) for a net loss - so the PE computes in
                   # f32r, but the HBM->SBUF transport is f16 (halves the
                   # ~39us DMA ramp) with a DVE copy upconverting on-chip.
QK_DT = F16        # q/k tiles feeding the scores matmul
PV_DT = BF16       # expS + v_ext feeding the PV matmul

# ---- DVE fast-exp constants -------------------------------------------------
# exp(x) ~= y0 * (1 + u*(C1Q + u*C2Q)), y0 = bitcast(int32(x*A32 + BETA)),
# u = 1 + mantissa_frac(y0). Quadratic fit of 2^(u-1)/u over [1,2); the
# fit's constant q0 is folded into BETA (shifts the exponent by log2 q0).
_Q2, _Q1, _Q0 = 0.22577846, -0.66678843, 1.43457818
A32C = float(np.float32(2.0**23 * np.log2(np.e)))
BETAC = float(np.float32((127.0 + np.log2(_Q0)) * 2.0**23))
C1Q = float(np.float32(_Q1 / _Q0))
C2Q = float(np.float32(_Q2 / _Q0))
MASKC = float(np.int32(0x007FFFFF).view(np.float32))


def _use_dve(t):
    # ~28 of 128 full tiles on DVE balances ScalarE/DVE (~111us each,
    # DVE also carries all evacuations + the f16->f32 input upconverts).
    # No DVE tiles in the first window: the DVE queue must stay clear for
    # the interleaved v-projection evacuations that early PV steps need.
    return t >= 16 and t % 8 in (2, 6)


_EXP2_OP = None


def _register_exp2_op():
    """Register the custom DVE op with concourse.dve_ops (documented
    extension point: append a DveOp to OPS). sha computed on the fly."""
    global _EXP2_OP
    if _EXP2_OP is not None:
        return _EXP2_OP
    from concourse import dve_ops as dops
    from concourse.dve_spec import AluOp, Bin, Spec, Src0, C0, C1, C2, One, lower
    from concourse.dve_uop import DveOpSpec

    name = "EXP2_CORR_ANT"
    for op in dops.OPS:
        if op.name == name:
            _EXP2_OP = op
            return op

    u = Bin(AluOp.BITWISE_OR, Bin(AluOp.BITWISE_AND, Src0, C0), One)
    body = Src0 * (One + u * (C2 + u * C1))

    def ref(in0, in1, s0, s1, imm2):
        bits = np.asarray(in0, np.float32).view(np.int32)
        mask = np.float32(s0).view(np.int32)
        oneb = np.float32(1.0).view(np.int32)
        uv = ((bits & mask) | oneb).view(np.float32)
        return in0 * (np.float32(1.0) + uv * (np.float32(imm2) + uv * np.float32(s1)))

    spec = Spec(body=body, reference=ref)
    row = dops._CUSTOM_DVE_ROW_BASE + len(dops.OPS)
    assert row < 0x20
    shas = {}
    for ver in ("v3", "v4"):
        try:
            uops = lower(spec, ver=ver)
            shas[ver] = DveOpSpec(
                name=name, opcode=row, uops=uops, rd1_en=False).sha(ver)
        except Exception:
            pass
    op = dops.DveOp(name, spec, subdim=False, uops_sha=shas)
    dops.OPS.append(op)
    dops._SUB_OPCODE_FOR_NAME[name] = row
    dops.CUSTOM_DVE_SPECS[name] = spec
    _EXP2_OP = op
    return op


_nc = None


def _build():
    global _nc
    if _nc is not None:
        return _nc
    exp2_op = _register_exp2_op()
    nc = bacc.Bacc("TRN2", target_bir_lowering=False, debug=False,
                   num_devices=NCORES)
    xt = nc.dram_tensor("xt", [N, D], F16, kind="ExternalInput").ap()
    wqk = nc.dram_tensor("wqk", [N, 4 * 128], F16,
                         kind="ExternalInput").ap()
    wv = nc.dram_tensor("wv", [N, HPC * P], F16, kind="ExternalInput").ap()
    bqk = nc.dram_tensor("bqk", [128, 4], F32, kind="ExternalInput").ap()
    o = nc.dram_tensor("o", [HPC, P + 1, D], F32, kind="ExternalOutput").ap()
    o_r = o.rearrange("h p d -> (h p) d")

    with tile.TileContext(nc) as tc:
        with tc.tile_pool(name="big", bufs=1) as big, \
             tc.tile_pool(name="es", bufs=8) as es, \
             tc.tile_pool(name="s32p", bufs=3) as s32p, \
             tc.tile_pool(name="obp", bufs=4) as obp:

            xt_t = big.tile([128, KT * D], PJ_DT, tag="xt")
            wqk_t = big.tile([128, KT * 512], PJ_DT, tag="wqk")
            wv_t = big.tile([128, KT * 256], PJ_DT, tag="wv")
            xt16 = big.tile([128, KT * D], F16, tag="xt16")
            wqk16 = big.tile([128, KT * 512], F16, tag="wqk16")
            wv16 = big.tile([128, KT * 256], F16, tag="wv16")
            bqk_t = big.tile([128, 4], F32, tag="bqk")
            scr = big.tile([128, 8], F32, tag="scr")
            qkT = big.tile([128, 4 * D], QK_DT, tag="qkT")
            vx4 = big.tile([128, JT, HPC, P + 1], PV_DT, tag="vx")

            def up(dst, src):
                # f16 -> f32r upconvert on DVE (2x mode)
                nc.vector.tensor_copy(dst, src)

            # DMA order = consumption order; every transfer is f16 and a
            # DVE copy upconverts right behind it. x ships in 512-col
            # pieces: piece 0 of every k-tile (plus the wqk pair-0 slices
            # and wv) is enough to start the q/k s0 chains, v[0..3], and
            # with them the whole attention pipeline, ~15us earlier than
            # waiting for all of x.
            nc.sync.dma_start(out=bqk_t[:], in_=bqk)
            for k in range(KT):
                nc.sync.dma_start(out=wqk16[:, k * 512:k * 512 + 256],
                                  in_=wqk[k * 128:(k + 1) * 128, 0:256])
                nc.sync.dma_start(out=xt16[:, k * D:k * D + 512],
                                  in_=xt[k * 128:(k + 1) * 128, 0:512])
                up(wqk_t[:, k * 512:k * 512 + 256],
                   wqk16[:, k * 512:k * 512 + 256])
                up(xt_t[:, k * D:k * D + 512], xt16[:, k * D:k * D + 512])
            for k in range(KT):
                nc.sync.dma_start(out=wv16[:, k * 256:(k + 1) * 256],
                                  in_=wv[k * 128:(k + 1) * 128, :])
                up(wv_t[:, k * 256:(k + 1) * 256],
                   wv16[:, k * 256:(k + 1) * 256])
            for s in range(1, IQ):
                for k in range(KT):
                    sl = slice(k * D + s * 512, k * D + (s + 1) * 512)
                    nc.sync.dma_start(
                        out=xt16[:, sl],
                        in_=xt[k * 128:(k + 1) * 128, s * 512:(s + 1) * 512])
                    up(xt_t[:, sl], xt16[:, sl])
            for k in range(KT):
                nc.sync.dma_start(out=wqk16[:, k * 512 + 256:(k + 1) * 512],
                                  in_=wqk[k * 128:(k + 1) * 128, 256:512])
                up(wqk_t[:, k * 512 + 256:(k + 1) * 512],
                   wqk16[:, k * 512 + 256:(k + 1) * 512])
            # ones columns for v_ext (v evac overwrites cols 0..63)
            nc.gpsimd.memset(vx4[:], 1.0)
            # Pre-warm the ScalarE exp table set during the DMA ramp.
            nc.scalar.activation(scr[0:1, 0:1], bqk_t[0:1, 0:1], EXP)

            from contextlib import ExitStack
            with tc.tile_pool(name="st", bufs=2, space="PSUM") as stp, \
                 tc.tile_pool(name="po", bufs=2, space="PSUM") as pop, \
                 ExitStack() as late_ctx:
                # pj (2 banks) is only needed while projection fillers run
                # (iterations < 48); it is closed mid-loop and its banks
                # are rededicated to a third st slot, deepening the
                # QK->exp->free pipeline for the rest of the kernel.
                pj_ctx = ExitStack()
                pjp = pj_ctx.enter_context(
                    tc.tile_pool(name="pj", bufs=2, space="PSUM"))

                def emit_qk_chunk(ms_list):
                    """k-outer chain(s) for qk-proj (m, s) chunks; paired
                    chunks pipeline with the xt DMA tail."""
                    pts = {}
                    for m, s in ms_list:
                        pts[(m, s)] = pjp.tile([128, 512], F32, tag="pj",
                                               name=f"pj{m}{s}")
                    for k in range(KT):
                        for m, s in ms_list:
                            nc.tensor.matmul(
                                pts[(m, s)][:],
                                wqk_t[:, k * 512 + m * 128:
                                      k * 512 + (m + 1) * 128],
                                xt_t[:, k * D + s * 512:k * D + (s + 1) * 512],
                                start=(k == 0), stop=(k == KT - 1))
                    for m, s in ms_list:
                        nc.vector.tensor_scalar_add(
                            qkT[:, m * D + s * 512:m * D + (s + 1) * 512],
                            pts[(m, s)][:], bqk_t[:, m:m + 1])

                def emit_v_chunk(j):
                    pt = pjp.tile([128, HPC, P], F32, tag="pj", name=f"pv{j}")
                    for k in range(KT):
                        nc.tensor.matmul(
                            pt[:],
                            xt_t[:, k * D + j * 128:k * D + j * 128 + 128],
                            wv_t[:, k * 256:(k + 1) * 256],
                            start=(k == 0), stop=(k == KT - 1))
                    nc.vector.tensor_copy(vx4[:, j, :, 0:P], pt[:])

                # Upfront: the minimum needed for attention step 0 -
                # q-pair0 s0 and k-pair0 s0 - finishing with the DMA tail.
                emit_qk_chunk([(0, 0), (1, 0)])

                # Everything else is a filler slotted into early steps.
                fillers = {}

                def add_filler(step, fn):
                    fillers.setdefault(step, []).append(fn)

                for j in range(JT):                  # v[j] needed at PV(j)
                    add_filler(j, lambda j=j: emit_v_chunk(j))
                for i, s in enumerate((1, 2, 3)):    # k-pair0 s1..3
                    add_filler(2 + 4 * i, lambda s=s: emit_qk_chunk([(1, s)]))
                for i, s in enumerate((1, 2, 3)):    # q-pair0 s1..3 (iq1+)
                    add_filler(13 + 12 * i, lambda s=s: emit_qk_chunk([(0, s)]))
                # pair-1 q/k chunks are needed only from iteration 64 on -
                # slot them into the exp-bound late stretch (where the PE
                # otherwise idles on st slots) instead of the PE-bound
                # early phase. m3 s-chunk s feeds QK(64+4s); m2 s-chunk s
                # feeds QK(64+16s).
                add_filler(52, lambda: emit_qk_chunk([(2, 0)]))
                for i in range(IQ):                  # k-pair1 s0..3
                    add_filler(56 + 4 * i, lambda s=i: emit_qk_chunk([(3, s)]))
                for i, fs in enumerate((72, 88, 104)):   # q-pair1 s1..3
                    add_filler(fs, lambda s=i + 1: emit_qk_chunk([(2, s)]))

                steps = [(p, iq, j) for p in range(NPAIR)
                         for iq in range(IQ) for j in range(JT)]
                NS = len(steps)
                sts, ets, ots = {}, {}, {}

                stp2 = [None]

                def emit_qk(s):
                    p, iq, j = steps[s]
                    qoff = (2 * p) * D
                    koff = (2 * p + 1) * D
                    # one tile per step: both half-slots free together, so
                    # the pair always issues back-to-back (concurrent rows)
                    # keep DVE-exp'd steps off the single-buffer st2 slot:
                    # its next reuse would wait on the slower 2-op DVE exp
                    if stp2[0] is not None and s % 3 == 2 and not _use_dve(s):
                        st = stp2[0].tile([128, 1024], F32, tag="st2",
                                          name="st")
                    else:
                        st = stp.tile([128, 1024], F32, tag="st", name="st")
                    for c0, r0 in ((0, 0), (512, 64)):
                        nc.tensor.matmul(
                            st[:, c0:c0 + 512],
                            qkT[r0:r0 + 64,
                                koff + j * 128:koff + (j + 1) * 128],
                            qkT[r0:r0 + 64,
                                qoff + iq * 512:qoff + (iq + 1) * 512],
                            start=True, stop=True)
                    sts[s] = st

                def emit_exp(s):
                    st = sts.pop(s)
                    et = es.tile([128, 1024], PV_DT, tag="et", name="et")
                    if _use_dve(s):
                        s32 = s32p.tile([128, 1024], I32, tag="s32",
                                        name="s32")
                        nc.vector.tensor_scalar(
                            out=s32[:], in0=st[:],
                            scalar1=A32C, scalar2=BETAC,
                            op0=mybir.AluOpType.mult,
                            op1=mybir.AluOpType.add)
                        nc.vector._custom_dve(
                            exp2_op, out=et[:],
                            in0=s32[:].bitcast(F32),
                            s0=MASKC, s1=C2Q, imm2=C1Q)
                    else:
                        nc.scalar.activation(et[:], st[:], EXP)
                    ets[s] = et

                def emit_pv(s):
                    p, iq, j = steps[s]
                    et = ets.pop(s)
                    if j == 0:
                        ots[(p, iq)] = (
                            pop.tile([P + 1, 512], F32, tag="po", name="otA"),
                            pop.tile([P + 1, 512], F32, tag="po", name="otB"))
                    otA, otB = ots[(p, iq)]
                    for c0, ot, h in ((0, otA, 2 * p), (512, otB, 2 * p + 1)):
                        nc.tensor.matmul(
                            ot[:], vx4[:, j, h, :], et[:, c0:c0 + 512],
                            start=(j == 0), stop=(j == JT - 1))
                    if j == JT - 1:
                        del ots[(p, iq)]
                        # evacuate the two accumulators on different
                        # engines so both ot banks free in parallel
                        for ot, h, eng in ((otA, 2 * p, nc.vector.tensor_copy),
                                           (otB, 2 * p + 1, nc.scalar.copy)):
                            ob = obp.tile([P + 1, 512], F32, tag="ob",
                                          name="ob")
                            eng(ob[:], ot[:])
                            nc.sync.dma_start(
                                out=o_r[h * (P + 1):(h + 1) * (P + 1),
                                        iq * 512:(iq + 1) * 512],
                                in_=ob[:])

                # iteration s: QK(s), exp(s-1), PV(s-4). exp trails QK by
                # one step so st slots free a full iteration before reuse;
                # PV trails exp by three so et semaphores fire well before
                # the PV LDWEIGHTS needs them (keeps weight-load pull-ahead
                # working on the PE) even when that exp ran on the slower
                # DVE path.
                for s in range(NS + 4):
                    if s == 110:
                        # all proj fillers done (last at 104): swap the pj
                        # banks for a third st slot for the filler-free tail
                        assert not fillers, f"fillers left: {sorted(fillers)}"
                        pj_ctx.close()
                        stp2[0] = late_ctx.enter_context(
                            tc.tile_pool(name="st2", bufs=1, space="PSUM"))
                    if s < NS:
                        for f in fillers.pop(s, ()):
                            f()
                        emit_qk(s)
                    if 1 <= s < NS + 1:
                        emit_exp(s - 1)
                    if s >= 4:
                        emit_pv(s - 4)
    nc.compile()
    _nc = nc
    return nc


def _np_dt(dt):
    if dt == BF16:
        return ml_dtypes.bfloat16
    if dt == mybir.dt.float16:
        return np.float16
    return np.float32


def _shard_inputs(x, W_qkv, b_qkv):
    pj = np.float16  # f16 transport; upconverted to f32r on-chip
    in_maps = []
    for c in range(NCORES):
        b = c // 4
        h0 = HPC * (c % 4)
        xT = np.ascontiguousarray(x[b].T).astype(pj)
        # wqk chunk layout: [q-pair0 | k-pair0 | q-pair1 | k-pair1], each
        # chunk = [head-even 64 cols | head-odd 64 cols]
        cols = []
        for p in range(NPAIR):
            cols.append(W_qkv[:, (h0 + 2 * p) * P:(h0 + 2 * p + 2) * P])
            cols.append(W_qkv[:, N + (h0 + 2 * p) * P:N + (h0 + 2 * p + 2) * P])
        wqk_m = np.ascontiguousarray(np.concatenate(cols, axis=1)).astype(pj)
        wv = np.ascontiguousarray(
            W_qkv[:, 2 * N + h0 * P:2 * N + (h0 + HPC) * P]).astype(pj)
        bcols = []
        for p in range(NPAIR):
            bcols.append(b_qkv[(h0 + 2 * p) * P:(h0 + 2 * p + 2) * P])
            bcols.append(b_qkv[N + (h0 + 2 * p) * P:N + (h0 + 2 * p + 2) * P])
        bqk = np.ascontiguousarray(np.stack(bcols, axis=1)).astype(np.float32)
        in_maps.append({"xt": xT, "wqk": wqk_m, "wv": wv, "bqk": bqk})
    return in_maps


def _assemble(results, b_qkv):
    out = np.empty((B, D, N), dtype=np.float32)
    for c in range(NCORES):
        b = c // 4
        h0 = HPC * (c % 4)
        oe = results[c]["o"]                      # (4, 65, 2048)
        att = oe[:, :P, :] / oe[:, P:P + 1, :]    # (4, 64, 2048)
        att = np.transpose(att, (0, 2, 1))        # (4, 2048, 64)
        for hl in range(HPC):
            h = h0 + hl
            bv = b_qkv[2 * N + h * P:2 * N + (h + 1) * P]
            out[b, h * 128:(h + 1) * 128, :] = \
                (att[hl] + bv[None, :]).reshape(128, N)
    return out


def _forward(in_maps, **kwargs):
    nc = _build()
    return run_bass_kernel_spmd(nc, in_maps, core_ids=list(range(NCORES)),
                                **kwargs)


def kernel(x, W_qkv, b_qkv):
    x = np.asarray(x, dtype=np.float32)
    W_qkv = np.asarray(W_qkv, dtype=np.float32)
    b_qkv = np.asarray(b_qkv, dtype=np.float32)
    in_maps = _shard_inputs(x, W_qkv, b_qkv)
    res = _forward(in_maps)
    return _assemble(res.results, b_qkv)
